# revision 20
# baseline (speedup 1.0000x reference)
"""Trainium2 Bass kernel for nn_DeChunkLayer (H-Net dechunk: EMA over chunks +
broadcast back to token positions).

Formulation: instead of (argsort -> EMA over M -> gather back to L), run ONE
first-order linear recurrence over the L-length token axis:
    a_l = mask_l ? (1 - p_l) : 1
    b_l = mask_l ? p_l * x[pbi_l] : 0        (pbi = cumsum(mask) - 1)
    H_l = a_l * H_{l-1} + b_l
Then out[l] = H_l exactly (at the m-th boundary H becomes ema[m]; in between it
holds). No argsort/compaction and no output gather; the only data-dependent
movement is the row gather x[pbi_l], done with the HW-accelerated dma_gather.

Per chunk of 128 positions the recurrence is solved with matmuls:
    out[i] = sum_{j<=i} exp(S_i - S_j) * s_j * x[pbi_j]  +  exp(S_i) * H_prev
(S = within-chunk inclusive cumsum of log a, s = mask * p). The chunk is laid
out REVERSED on the output partitions (row 0 = chunk end) so the inter-chunk
carry H_c can be read straight out of the main matmul's PSUM row 0 before the
carry rank-1 matmul accumulates.

bf16 datapath: x is gathered as bf16 (half the DMA traffic), the weight
matrices and carry operands are bf16 (1 PE cycle/row instead of 4 for fp32),
and the output is written as bf16 (host upcasts). The precision-critical
pieces (S cumsum, exp biases, per-chunk decay g) stay fp32.

Sharded over batch: core b handles batch row b.
"""

import numpy as np

import concourse.bass as bass
import concourse.tile as tile
from concourse import bacc, mybir

F32 = mybir.dt.float32
BF16 = mybir.dt.bfloat16
I16 = mybir.dt.int16
AX = mybir.AluOpType
ACT = mybir.ActivationFunctionType

# Problem constants (hardcoded per contract)
B, L, D, M = 8, 8192, 1024, 2048
EPS = 1e-4
N_CORES = 8


def build_program(L_=L, D_=D, M_=M, reps=1, GB=2, xg_bufs=3, osb_bufs=6):
    """Build the per-core Bass program. Returns (nc, names dict)."""
    CH = 128                       # chunk length (= matmul K)
    NCH = L_ // CH                 # number of chunks
    NF = L_ // 16                  # wrapped-16 index columns
    NG = NCH // GB                 # GB chunks per batched gather
    assert NCH * CH == L_ and NG * GB == NCH
    NSPL = 512                     # matmul free-dim split (one PSUM bank)
    NH = D_ // NSPL

    from contextlib import ExitStack

    nc = bacc.Bacc(None, target_bir_lowering=False, debug=False)
    with tile.TileContext(nc) as tc, ExitStack() as ctx:
        dram = ctx.enter_context(tc.tile_pool(name="dram", bufs=1, space="DRAM"))
        x_d = dram.tile([M_, D_], BF16, kind="ExternalInput")
        p_d = dram.tile([NCH, CH], F32, kind="ExternalInput")
        m_d = dram.tile([NCH, CH], F32, kind="ExternalInput")
        m16_d = dram.tile([16, NF], F32, kind="ExternalInput")
        ident_d = dram.tile([128, 128], F32, kind="ExternalInput")
        rev_d = dram.tile([128, 128], F32, kind="ExternalInput")
        mnegr_d = dram.tile([128, 128], F32, kind="ExternalInput")
        le16_d = dram.tile([16, 16], F32, kind="ExternalInput")
        gt16_d = dram.tile([16, 16], F32, kind="ExternalInput")
        rep16_d = dram.tile([16, 128], F32, kind="ExternalInput")
        out_d = dram.tile([L_, D_], BF16, kind="ExternalOutput")

        setup = ctx.enter_context(tc.tile_pool(name="setup", bufs=1))
        bsp = ctx.enter_context(tc.tile_pool(name="bsp", bufs=2, space="PSUM"))
        xgp = ctx.enter_context(tc.tile_pool(name="xgp", bufs=xg_bufs))
        ttp = ctx.enter_context(tc.tile_pool(name="ttp", bufs=3))
        esp = ctx.enter_context(tc.tile_pool(name="esp", bufs=3))
        hp = ctx.enter_context(tc.tile_pool(name="hp", bufs=4))
        osb = ctx.enter_context(tc.tile_pool(name="osb", bufs=osb_bufs))

        # ---------------- setup ----------------
        ident = setup.tile([128, 128], F32)
        nc.sync.dma_start(out=ident[:], in_=ident_d[:])
        rev128 = setup.tile([128, 128], F32)
        nc.sync.dma_start(out=rev128[:], in_=rev_d[:])
        mnegr = setup.tile([128, 128], F32)
        nc.sync.dma_start(out=mnegr[:], in_=mnegr_d[:])
        le16 = setup.tile([16, 16], F32)
        nc.sync.dma_start(out=le16[:], in_=le16_d[:])
        gt16 = setup.tile([16, 16], F32)
        nc.sync.dma_start(out=gt16[:], in_=gt16_d[:])
        rep16 = setup.tile([16, 128], F32)
        nc.sync.dma_start(out=rep16[:], in_=rep16_d[:])

        praw = setup.tile([NCH, CH], F32)
        nc.sync.dma_start(out=praw[:], in_=p_d[:])
        mk = setup.tile([NCH, CH], F32)
        nc.sync.dma_start(out=mk[:], in_=m_d[:])
        m16 = setup.tile([16, NF], F32)
        nc.sync.dma_start(out=m16[:], in_=m16_d[:])

        ones_r = setup.tile([NCH, CH], F32)
        nc.vector.memset(ones_r[:], 1.0)
        ones16 = setup.tile([16, NF], F32)
        nc.vector.memset(ones16[:], 1.0)

        pc = setup.tile([NCH, CH], F32)
        nc.vector.tensor_scalar(out=pc[:], in0=praw[:], scalar1=EPS,
                                scalar2=1.0 - EPS, op0=AX.max, op1=AX.min)
        sg = setup.tile([NCH, CH], F32)
        nc.vector.tensor_tensor(out=sg[:], in0=pc[:], in1=mk[:], op=AX.mult)
        q = setup.tile([NCH, CH], F32)
        nc.vector.tensor_scalar(out=q[:], in0=pc[:], scalar1=-1.0,
                                scalar2=1.0, op0=AX.mult, op1=AX.add)
        lnq = setup.tile([NCH, CH], F32)
        nc.scalar.activation(out=lnq[:], in_=q[:], func=ACT.Ln)
        loga = setup.tile([NCH, CH], F32)
        nc.vector.tensor_tensor(out=loga[:], in0=lnq[:], in1=mk[:], op=AX.mult)

        # within-chunk inclusive cumsum of log(a) (along free dim)
        s_i = setup.tile([NCH, CH], F32)
        nc.vector.tensor_tensor_scan(out=s_i[:], data0=ones_r[:], data1=loga[:],
                                     initial=0.0, op0=AX.mult, op1=AX.add)

        # ---- gather indices in wrapped-16 int16 layout (for dma_gather) ----
        # per-phase inclusive cumsum of the wrapped mask, then combine phases:
        # incl[16f+p] = sum_{p'<=p} C16[p'][f] + sum_{p'>p} C16[p'][f-1]
        c16 = setup.tile([16, NF], F32)
        nc.vector.tensor_tensor_scan(out=c16[:], data0=ones16[:], data1=m16[:],
                                     initial=0.0, op0=AX.mult, op1=AX.add)
        idx16 = setup.tile([128, NF], I16)
        with tc.tile_pool(name="bsps", bufs=1, space="PSUM") as bsps:
            pbi16_ps = bsps.tile([16, NF], F32, tag="bs16")
            nc.tensor.matmul(out=pbi16_ps[0:16, 0:NF], lhsT=le16[:],
                             rhs=c16[:], start=True, stop=False,
                             skip_group_check=True)
            nc.tensor.matmul(out=pbi16_ps[0:16, 1:NF], lhsT=gt16[:],
                             rhs=c16[0:16, 0:NF - 1],
                             start=False, stop=True, skip_group_check=True)
            pbi16 = setup.tile([16, NF], F32)
            nc.vector.tensor_scalar_add(out=pbi16[:],
                                        in0=pbi16_ps[0:16, 0:NF],
                                        scalar1=-1.0)
            idxrep_ps = bsps.tile([128, NF], F32, tag="bs16r")
            nc.tensor.matmul(out=idxrep_ps[0:128, 0:NF], lhsT=rep16[:],
                             rhs=pbi16[:], start=True, stop=True)
            nc.vector.tensor_copy(out=idx16[:], in_=idxrep_ps[0:128, 0:NF])

        # transposed per-chunk columns: s (scale), S and -S
        sT_ps = bsp.tile([128, 128], F32, tag="bs")
        nc.tensor.transpose(out=sT_ps[0:CH, 0:NCH], in_=sg[:],
                            identity=ident[0:NCH, 0:NCH])
        sT = setup.tile([CH, NCH], F32)
        nc.vector.tensor_copy(out=sT[:], in_=sT_ps[0:CH, 0:NCH])
        ST_ps = bsp.tile([128, 128], F32, tag="bs")
        nc.tensor.transpose(out=ST_ps[0:CH, 0:NCH], in_=s_i[:],
                            identity=ident[0:NCH, 0:NCH])
        ST = setup.tile([CH, NCH], F32)
        nc.vector.tensor_copy(out=ST[:], in_=ST_ps[0:CH, 0:NCH])
        negST = setup.tile([CH, NCH], F32)
        nc.vector.tensor_scalar_mul(out=negST[:], in0=ST[:], scalar1=-1.0)

        # per-chunk total decay g_c = exp(Z_c) as a row (free-indexed)
        z_rowT = bsp.tile([128, 128], F32, tag="bs")
        nc.tensor.transpose(out=z_rowT[0:1, 0:NCH], in_=s_i[:, CH - 1:CH],
                            identity=ident[0:NCH, 0:NCH])
        g_row = setup.tile([1, NCH], F32)
        nc.scalar.activation(out=g_row[:], in_=z_rowT[0:1, 0:NCH], func=ACT.Exp)

        # ---------------- main loop ----------------
        outp = ctx.enter_context(tc.tile_pool(name="outp", bufs=3,
                                              space="PSUM"))
        for _rep in range(reps):
            h_tiles = {}
            xg_tiles = {}
            for c in range(NCH):
                g = c // GB
                if c % GB == 0:
                    xg = xgp.tile([CH, GB * D_], BF16, tag="xg")
                    nc.gpsimd.dma_gather(
                        out_ap=xg[:].rearrange("p (a b) -> p a b", a=GB),
                        in_ap=x_d[:],
                        idxs_ap=idx16[:, 8 * GB * g:8 * GB * (g + 1)],
                        num_idxs=GB * CH, num_idxs_reg=GB * CH,
                        elem_size=D_)
                    xg_tiles[g] = xg
                xg_c = xg_tiles[g][:, (c % GB) * D_:(c % GB + 1) * D_]

                # Sbc[j, i'] = S_{127-i'} (broadcast along partitions):
                # PE transpose of the free-broadcast S column against the
                # anti-diagonal permutation (reversed output layout)
                sbc = bsp.tile([128, 128], F32, tag="bs")
                nc.tensor.transpose(out=sbc[0:CH, 0:CH],
                                    in_=ST[:, c:c + 1].to_broadcast([CH, CH]),
                                    identity=rev128[0:CH, 0:CH])
                ttq = ttp.tile([CH, CH], F32, tag="ttq")
                nc.vector.tensor_tensor(out=ttq[:], in0=sbc[0:CH, 0:CH],
                                        in1=mnegr[0:CH, 0:CH], op=AX.add)
                ttm = ttp.tile([CH, CH], BF16, tag="ttm")
                nc.scalar.activation(out=ttm[:], in_=ttq[:], func=ACT.Exp,
                                     bias=negST[:, c:c + 1])
                esr = esp.tile([1, CH], BF16, tag="es")
                nc.scalar.activation(out=esr[:], in_=sbc[0:1, 0:CH],
                                     func=ACT.Exp)
                tts = ttp.tile([CH, CH], BF16, tag="tts")
                nc.gpsimd.tensor_scalar_mul(out=tts[:], in0=ttm[:],
                                            scalar1=sT[:, c:c + 1])

                # main matmul (reversed rows; row 0 = chunk end, sans carry)
                op_t = outp.tile([128, D_], F32, tag="op")
                for h in range(NH):
                    sl = slice(h * NSPL, (h + 1) * NSPL)
                    nc.tensor.matmul(out=op_t[0:CH, sl], lhsT=tts[:],
                                     rhs=xg_c[:, sl], start=True, stop=True)

                # H chain: H_c = g_c * H_{c-1} + psum_row0 (y_c)
                h_t = hp.tile([1, D_], BF16, tag="h")
                if c == 0:
                    nc.vector.tensor_copy(out=h_t[:], in_=op_t[0:1, :])
                else:
                    nc.vector.scalar_tensor_tensor(
                        out=h_t[:], in0=h_tiles[c - 1][:],
                        scalar=g_row[0:1, c:c + 1], in1=op_t[0:1, :],
                        op0=AX.mult, op1=AX.add)
                h_tiles[c] = h_t

                # carry accumulation (after the chain's psum read)
                if c > 0:
                    for h in range(NH):
                        sl = slice(h * NSPL, (h + 1) * NSPL)
                        nc.tensor.matmul(out=op_t[0:CH, sl], lhsT=esr[:],
                                         rhs=h_tiles[c - 1][0:1, sl],
                                         start=False, stop=True,
                                         skip_group_check=True)

                o_sb = osb.tile([CH, D_], BF16, tag="osb")
                nc.scalar.activation(out=o_sb[:], in_=op_t[0:CH, :],
                                     func=ACT.Copy)
                # rows land block-reversed; the host unshard flips them back
                nc.sync.dma_start(out=out_d[c * CH:(c + 1) * CH, :],
                                  in_=o_sb[:])

    nc.compile()
    names = dict(x=x_d.name, p=p_d.name, m=m_d.name, m16=m16_d.name,
                 ident=ident_d.name, rev=rev_d.name, mnegr=mnegr_d.name,
                 le16=le16_d.name, gt16=gt16_d.name, rep16=rep16_d.name,
                 out=out_d.name)
    return nc, names


def make_consts():
    ident = np.eye(128, dtype=np.float32)
    rev = np.eye(128, dtype=np.float32)[::-1].copy()
    jj = np.arange(128)
    # reversed triangular mask: out-row i' holds position (127 - i')
    mnegr = np.where(jj[:, None] > 127 - jj[None, :], -1e30, 0.0).astype(
        np.float32)
    p16 = np.arange(16)
    le16 = (p16[:, None] <= p16[None, :]).astype(np.float32)
    gt16 = (p16[:, None] > p16[None, :]).astype(np.float32)
    rep16 = (p16[:, None] == (np.arange(128) % 16)[None, :]).astype(np.float32)
    return dict(ident=ident, rev=rev, mnegr=mnegr, le16=le16, gt16=gt16,
                rep16=rep16)


_CACHE = {}


def _get_program():
    if "prog" not in _CACHE:
        _CACHE["prog"] = build_program()
    return _CACHE["prog"]


def per_core_inputs(names, hidden_b, bprob_b, mask_b, L_=L):
    import ml_dtypes

    NCH = L_ // 128
    NF = L_ // 16
    cs = make_consts()
    mf = mask_b.astype(np.float32)
    return {
        names["x"]: np.ascontiguousarray(
            np.asarray(hidden_b).astype(ml_dtypes.bfloat16)),
        names["p"]: np.ascontiguousarray(bprob_b[:, 1].reshape(NCH, 128)),
        names["m"]: np.ascontiguousarray(mf.reshape(NCH, 128)),
        names["m16"]: np.ascontiguousarray(mf.reshape(NF, 16).T),
        names["ident"]: cs["ident"],
        names["rev"]: cs["rev"],
        names["mnegr"]: cs["mnegr"],
        names["le16"]: cs["le16"],
        names["gt16"]: cs["gt16"],
        names["rep16"]: cs["rep16"],
    }


def kernel(hidden_states, boundary_prob, boundary_mask):
    from concourse import bass_utils

    nc, names = _get_program()

    hidden_states = np.asarray(hidden_states, dtype=np.float32)
    boundary_prob = np.asarray(boundary_prob, dtype=np.float32)
    boundary_mask = np.asarray(boundary_mask)

    in_maps = [per_core_inputs(names, hidden_states[b], boundary_prob[b],
                               boundary_mask[b]) for b in range(B)]
    res = bass_utils.run_bass_kernel_spmd(nc, in_maps,
                                          core_ids=list(range(N_CORES)))
    out = np.stack([np.asarray(res.results[b][names["out"]])
                    for b in range(B)], axis=0).astype(np.float32)
    # un-flip the per-chunk row reversal (device writes chunk rows reversed)
    out = out.reshape(B, L // 128, 128, D)[:, :, ::-1, :].reshape(B, L, D)
    return np.ascontiguousarray(out, dtype=np.float32)
